# revision 29
# baseline (speedup 1.0000x reference)
"""BitLinear inference kernel for Trainium2 (8 NeuronCores, column-parallel).

Math (per reference):
  s[t]   = max(|x[t,:]|) clipped to >= 1e-5          (per-token scale)
  xq     = round(x / s * 127)  (round-half-even)      (int values in [-127,127])
  out    = (xq @ w_ternary.T) * (s * weight_scale / 127)

The integer matmul xq @ w.T is EXACT in bf16 x bf16 -> fp32 PSUM:
xq in [-127,127] and w in {-1,0,1} are exactly representable in bf16,
products are exact, and partial sums are < 2^24 so fp32 accumulation is
exact. Per-token dequant scale is applied to the fp32 PSUM output.

Sharding: column-parallel. weight rows (out_features) are sharded 8 ways;
x is replicated; outputs are concatenated on host along out_features.
The weight shard is shipped host-transposed ([in_f, of_shard], still int32)
so the contraction dim lands on SBUF partitions with contiguous DMA.

Per-core pipeline, per 128-token tile:
  DMA   x tile in (2 halves), per-tile DVE quant (abs-max reduce,
        reciprocal, mult+magic-add, magic-sub -> bf16),
  DMA   xbar transpose SBUF->SBUF (bf16) into [128, 32, 128] lhsT chunks,
  PE    32 LDW+128 matmuls (N=512) accumulating [128 tok, 2048 of] fp32
        across 2 double-buffered PSUM tiles (8 banks),
  ACT   per-token-scale eviction (activation Copy, scale=[128,1] AP),
  DMA   store.
Weights are DMA'd int32 once at start and cast to bf16 on GPSIMD.

Measured on trn2 (slope over a hardware For_i repeat loop, R=1 vs 1025):
1.824 ms/pass per core = 97% of the 78.6 TF/s bf16 PE roofline
(1.1 TFLOP total / 8 cores). Correctness vs the fp32 jax reference:
norm relative error 2.3e-05 (from inv=127*(1/s) vs the reference's
x/s*127 double-rounding; the integer matmul itself is exact).
"""

import numpy as np

import concourse.bass as bass
import concourse.mybir as mybir
import concourse.tile as tile
from concourse import bacc

P = 128
MAGIC = 12582912.0  # 1.5 * 2**23: (v + MAGIC) - MAGIC == round-half-even(v) for |v|<=2^21

# problem shapes (hardcoded per contract)
B, S, IN_F, OUT_F = 4, 2048, 4096, 16384
N_CORES = 8
TOKENS = B * S
OF_SHARD = OUT_F // N_CORES


def build_program(tokens=TOKENS, in_f=IN_F, of=OF_SHARD, n_devices=N_CORES,
                  debug=False, ns=512, reps=1, timing=False, variant="full",
                  quant_on_act=False, deep=False):
    """Build the SPMD single-core program. Returns the compiled Bacc object.

    timing=True makes the big tensors internal (nothing shipped over the
    wire) and adds a tiny external in/out pair; reps>1 wraps the token loop
    in a hardware For_i so per-iteration time can be measured as a slope.
    """
    TT = tokens // P      # token tiles
    KC = in_f // P        # contraction chunks
    NOF = of // ns        # psum column slices
    XH = in_f // 2        # x staged in halves to save SBUF

    nc = bacc.Bacc("TRN2", target_bir_lowering=False, debug=debug,
                   num_devices=n_devices)

    big_kind = "Internal" if timing else "ExternalInput"
    xf = nc.dram_tensor("x", [tokens, in_f], mybir.dt.float32,
                        kind=big_kind).ap()
    wt = nc.dram_tensor("wt", [in_f, of], mybir.dt.int32,
                        kind=big_kind).ap()
    ws = nc.dram_tensor("ws", [P, 1], mybir.dt.float32,
                        kind="ExternalInput").ap()
    out = nc.dram_tensor(
        "out", [tokens, of], mybir.dt.float32,
        kind="Internal" if timing else "ExternalOutput").ap()
    tiny = None
    if timing:
        tiny = nc.dram_tensor("tiny", [P, 1], mybir.dt.float32,
                              kind="ExternalOutput").ap()
    xqd = None
    if variant == "drtr":
        xqd = nc.dram_tensor("xq_scratch", [tokens // P, P, in_f],
                             mybir.dt.bfloat16, kind="Internal").ap()

    xf3 = xf.rearrange("(tt p) f -> tt p f", p=P)
    wt3 = wt.rearrange("(kc p) o -> kc p o", p=P)
    out3 = out.rearrange("(tt p) o -> tt p o", p=P)

    petr = variant == "petr"
    with tile.TileContext(nc) as tc:
        with (
            tc.tile_pool(name="consts", bufs=1) as consts,
            tc.tile_pool(name="wpool", bufs=1) as wpool,
            tc.tile_pool(name="stage", bufs=3 if deep else 2) as stage,
            tc.tile_pool(name="xqp", bufs=2 if deep else 1) as xqp,
            tc.tile_pool(name="xqtp", bufs=3 if deep else 2) as xqtp,
            tc.tile_pool(name="outp", bufs=2) as outp,
            tc.tile_pool(name="scal", bufs=3) as scal,
            tc.tile_pool(name="psum", bufs=1 if petr else 2,
                         space="PSUM") as psum,
            tc.tile_pool(name="psum_tr", bufs=2, space="PSUM") as psum_tr,
        ):
            c127 = consts.tile([P, 1], mybir.dt.float32)
            nc.vector.memset(c127[:], 127.0)
            wsb = consts.tile([P, 1], mybir.dt.float32)
            nc.sync.dma_start(wsb[:], ws[:])
            identity = None
            if petr:
                from concourse.masks import make_identity
                identity = consts.tile([P, P], mybir.dt.bfloat16)
                make_identity(nc, identity[:])

            # tile 0's x loads first so they land at the DMA queue heads
            pre_x = []
            if reps == 1 and variant not in ("mm",):
                for h in range(2):
                    xt = stage.tile([P, XH], mybir.dt.float32, tag="stage",
                                    name=f"prex{h}")
                    nc.sync.dma_start(xt[:], xf3[0][:, h * XH:(h + 1) * XH])
                    pre_x.append(xt)

            # ---- weights: int32 [in_f, of] -> bf16 chunks [P, of] resident
            wks = []
            for k in range(KC):
                st = stage.tile([P, of], mybir.dt.int32, tag="wstage",
                                name="wst")
                nc.sync.dma_start(st[:], wt3[k])
                wk = wpool.tile([P, of], mybir.dt.bfloat16, tag=f"wk{k}")
                nc.gpsimd.tensor_copy(wk[:], st[:])
                wks.append(wk)

            # mm-only variant: constant stationary tile + scale, no quant path
            cxqt = cfs = None
            if variant == "mm":
                cxqt = consts.tile([P, KC, P], mybir.dt.bfloat16)
                nc.vector.memset(cxqt[:], 1.0)
                cfs = consts.tile([P, 1], mybir.dt.float32)
                nc.vector.memset(cfs[:], 1.0)

            # ---- main loop over token tiles
            def token_loop():
                for t in range(TT):
                    if variant == "mm":
                        mm_tile(t, cxqt, cfs)
                    else:
                        token_tile(t)

            def mm_tile(t, xqt, fs):
                ps = psum.tile([P, of], mybir.dt.float32)
                for k in range(KC):
                    for n in range(NOF):
                        nc.tensor.matmul(
                            ps[:, n * ns:(n + 1) * ns],
                            xqt[:, k, :],
                            wks[k][:, n * ns:(n + 1) * ns],
                            start=(k == 0), stop=(k == KC - 1))
                ot = outp.tile([P, of], mybir.dt.float32, name="ot_mm")
                for n in range(NOF):
                    nc.scalar.mul(ot[:, n * ns:(n + 1) * ns],
                                  ps[:, n * ns:(n + 1) * ns], fs[:])
                nc.sync.dma_start(out3[t], ot[:])

            def token_tile(t):
                # per-tile scalar vectors packed into one tile (SBUF slots
                # pad to 4KB/partition, so one tag instead of four)
                scv = scal.tile([P, 8], mybir.dt.float32, tag="scv",
                                name="scv")
                sc2 = scv[:, 0:2]
                s = scv[:, 2:3]
                inv = scv[:, 3:4]
                fs = scv[:, 4:5]
                # load x tile in halves, quantize
                xh = [None, None]
                for h in range(2):
                    if t == 0 and reps == 1 and pre_x:
                        xh[h] = pre_x[h]
                    else:
                        xh[h] = stage.tile([P, XH], mybir.dt.float32,
                                           tag="stage", name=f"xh{h}")
                        nc.sync.dma_start(xh[h][:],
                                          xf3[t][:, h * XH:(h + 1) * XH])
                    nc.vector.tensor_reduce(
                        sc2[:, h:h + 1], xh[h][:], axis=mybir.AxisListType.X,
                        op=mybir.AluOpType.max, apply_absolute_value=True)
                nc.vector.tensor_reduce(
                    s[:], sc2[:], axis=mybir.AxisListType.X,
                    op=mybir.AluOpType.max)
                nc.vector.tensor_scalar_max(s[:], s[:], 1e-5)
                nc.vector.reciprocal(inv[:], s[:])
                nc.vector.tensor_scalar_mul(inv[:], inv[:], 127.0)
                nc.vector.tensor_scalar(fs[:], s[:], wsb[:], 1.0 / 127.0,
                                        op0=mybir.AluOpType.mult,
                                        op1=mybir.AluOpType.mult)
                xq = xqp.tile([P, in_f], mybir.dt.bfloat16)
                for h in range(2):
                    xqs = xq[:, h * XH:(h + 1) * XH]
                    if quant_on_act:
                        nc.scalar.activation(
                            xh[h][:], xh[h][:],
                            mybir.ActivationFunctionType.Copy,
                            bias=MAGIC, scale=inv[:])
                        nc.vector.tensor_scalar(xqs, xh[h][:], MAGIC, None,
                                                op0=mybir.AluOpType.subtract)
                    else:
                        nc.vector.tensor_scalar(xh[h][:], xh[h][:], inv[:],
                                                MAGIC,
                                                op0=mybir.AluOpType.mult,
                                                op1=mybir.AluOpType.add)
                        nc.vector.tensor_scalar(xqs, xh[h][:], MAGIC, None,
                                                op0=mybir.AluOpType.subtract)

                # transpose xq [P, in_f] -> per-chunk [P, P] tiles
                if petr:
                    # PE transpose: xq_chunk.T @ I into PSUM, ACT copy to SBUF
                    xqtc = []
                    for k in range(KC):
                        pt = psum_tr.tile([P, P], mybir.dt.float32,
                                          tag="pt", name="pt")
                        nc.tensor.matmul(pt[:], xq[:, k * P:(k + 1) * P],
                                         identity[:], start=True, stop=True)
                        xc = xqtp.tile([P, P], mybir.dt.bfloat16,
                                       tag=f"xqt{k}", name=f"xqt{k}")
                        nc.scalar.copy(xc[:], pt[:])
                        xqtc.append(xc)
                    lhs = lambda k: xqtc[k][:]
                elif variant == "drtr":
                    nc.sync.dma_start(xqd[t], xq[:])
                    xqt = xqtp.tile([P, KC, P], mybir.dt.bfloat16)
                    nc.sync.dma_start_transpose(xqt[:], xqd[t])
                    lhs = lambda k: xqt[:, k, :]
                else:
                    xqt = xqtp.tile([P, KC, P], mybir.dt.bfloat16)
                    nc.sync.dma_start_transpose(xqt[:], xq[:])
                    lhs = lambda k: xqt[:, k, :]

                if variant == "qt":
                    # consume xqt without matmuls: store half of it
                    nc.sync.dma_start(out3[t], xqt[:, :KC // 2, :])
                    return

                # matmul: psum[tok, of] += xqt[k].T @ wk[k]
                ps = psum.tile([P, of], mybir.dt.float32)
                for k in range(KC):
                    for n in range(NOF):
                        nc.tensor.matmul(
                            ps[:, n * ns:(n + 1) * ns],
                            lhs(k),
                            wks[k][:, n * ns:(n + 1) * ns],
                            start=(k == 0), stop=(k == KC - 1))

                # evict with per-token scale, then store
                ot = outp.tile([P, of], mybir.dt.float32)
                for n in range(NOF):
                    nc.scalar.mul(ot[:, n * ns:(n + 1) * ns],
                                  ps[:, n * ns:(n + 1) * ns], fs[:])
                nc.sync.dma_start(out3[t], ot[:])

            if reps == 1:
                token_loop()
            else:
                with tc.For_i(0, reps, 1):
                    token_loop()
            if timing:
                nc.sync.dma_start(tiny[:], wsb[:])

    nc.compile()
    return nc


_CACHED = {}


def _get_program():
    if "nc" not in _CACHED:
        _CACHED["nc"] = build_program()
    return _CACHED["nc"]


def make_in_maps(x, weight_ternary, weight_scale):
    xf = np.ascontiguousarray(np.asarray(x).reshape(TOKENS, IN_F),
                              dtype=np.float32)
    wsb = np.full((P, 1), np.float32(np.asarray(weight_scale).reshape(-1)[0]),
                  dtype=np.float32)
    in_maps = []
    for c in range(N_CORES):
        shard = np.asarray(weight_ternary)[c * OF_SHARD:(c + 1) * OF_SHARD, :]
        wt_t = np.ascontiguousarray(shard.T).astype(np.int32)  # [IN_F, OF_SHARD]
        in_maps.append({"x": xf, "wt": wt_t, "ws": wsb})
    return in_maps


def gather_out(results):
    full = np.empty((TOKENS, OUT_F), dtype=np.float32)
    for c in range(N_CORES):
        full[:, c * OF_SHARD:(c + 1) * OF_SHARD] = results[c]["out"]
    return full.reshape(B, S, OUT_F)


def kernel(x, weight_ternary, weight_scale):
    from concourse.bass_utils import run_bass_kernel_spmd

    nc = _get_program()
    in_maps = make_in_maps(x, weight_ternary, weight_scale)
    try:
        res = run_bass_kernel_spmd(nc, in_maps, list(range(N_CORES)))
    except Exception:
        # transient device/transport flakes: retry once
        import time as _time
        _time.sleep(5)
        res = run_bass_kernel_spmd(nc, in_maps, list(range(N_CORES)))
    return gather_out(res.results)
